# revision 9
# baseline (speedup 1.0000x reference)
"""Trainium2 kernel for DWTFeatureModel.

Model: 3-level db4 DWT along time (256 -> 276 coeffs, reflect padding) for
each of B*64 channels, then a Conv3d whose kernel spans the whole
(276, 8, 8) volume (== full contraction to 64 features), bias, LeakyReLU.

The DWT is linear, so dwt(sig) = sig @ M for a fixed (256, 276) analysis
matrix M built from the db4 filter bank. The whole model then collapses to

    out[b, f] = leaky(sum_{s,hw} x[b, s, hw] * Weff[s, hw, f] + bias[f])
    Weff[s, hw, f] = sum_t M[s, t] * W[f, t, hw]

Device kernel (per core, pure batch-data-parallel over 8 cores):
  phase 1 (fold):  Weff = M^T-contraction of the replicated conv weight,
                   computed on the tensor engine in fp32 (exact), laid out
                   directly as the stationary operands of phase 2.
  phase 2 (main):  out^T = Weff^T @ x^T as 128 accumulating matmuls
                   (K=128 each, N=256 batch columns) in fp32r, streaming
                   x from HBM (16 MB/core) in 2 MB double-buffered tiles.
  epilogue:        + bias, LeakyReLU via max(y, 0.02*y), DMA out.

Host side only shards/permutes inputs (x^T per core) and transposes the
(64, 256) per-core outputs back.
"""

from contextlib import ExitStack

import numpy as np

import concourse.bass as bass
import concourse.tile as tile
from concourse import bacc, mybir
from concourse.bass_utils import run_bass_kernel_spmd

# pywt db4 analysis filters (identical constants to the model definition)
DEC_LO = [-0.010597401784997278, 0.032883011666982945, 0.030841381835986965,
          -0.18703481171888114, -0.02798376941698385, 0.6308807679295904,
          0.7148465705525415, 0.23037781330885523]
DEC_HI = [-0.23037781330885523, 0.7148465705525415, -0.6308807679295904,
          -0.02798376941698385, 0.18703481171888114, 0.030841381835986965,
          -0.032883011666982945, -0.010597401784997278]

B, T, F, TDWT = 2048, 256, 64, 276
J, L = 3, 8
NEG_SLOPE = 0.02
NCORES = 8
BC = B // NCORES          # 256 batches per core
G = 128                   # contraction chunks of 128 (= 2 s-blocks x 64 hw)
XG = 16                   # contraction chunks per streamed x tile (2 MB)
TCH = [(0, 128), (128, 128), (256, 20)]  # t-chunks of the 276 DWT coeffs


def _build_dwt_matrix():
    """M (T, TDWT) with dwt(sig) = sig @ M, matching the reference's
    multi-level reflect-padded strided cross-correlation."""
    h_lo = np.array(DEC_LO, np.float64)[::-1]
    h_hi = np.array(DEC_HI, np.float64)[::-1]
    lo = np.eye(T, dtype=np.float64)
    his = []
    for _ in range(J):
        n = lo.shape[-1]
        outsize = (n + L - 1) // 2
        p = 2 * (outsize - 1) - n + L
        xp = np.pad(lo, ((0, 0), (p // 2, (p + 1) // 2)), mode="reflect")
        idx = np.arange(outsize)[:, None] * 2 + np.arange(L)[None, :]
        win = xp[:, idx]
        his.append(win @ h_hi)
        lo = win @ h_lo
    return np.concatenate([lo] + his, axis=-1)  # (256, 276)


def _emit(ctx, tc, xt, wt, dm, bi, outT):
    nc = tc.nc
    f32 = mybir.dt.float32
    f32r = mybir.dt.float32r

    const_pool = ctx.enter_context(tc.tile_pool(name="const", bufs=1))
    weff_pool = ctx.enter_context(tc.tile_pool(name="weff", bufs=1))
    xpool = ctx.enter_context(tc.tile_pool(name="x", bufs=4))
    fold_ps = ctx.enter_context(tc.tile_pool(name="foldps", bufs=2, space="PSUM"))
    out_ps = ctx.enter_context(tc.tile_pool(name="outps", bufs=1, space="PSUM"))
    opool = ctx.enter_context(tc.tile_pool(name="osb", bufs=1))

    # ---- constants
    wt_sb, dm_sb = [], []
    for ti, (t0, tsz) in enumerate(TCH):
        w = const_pool.tile([tsz, 64 * F], f32, tag=f"wt{ti}")
        nc.sync.dma_start(w[:], wt[t0:t0 + tsz, :])
        wt_sb.append(w)
        d = const_pool.tile([tsz, T], f32, tag=f"dm{ti}")
        nc.sync.dma_start(d[:], dm[t0:t0 + tsz, :])
        dm_sb.append(d)
    bias = const_pool.tile([F, 1], f32, tag="bias")
    nc.sync.dma_start(bias[:], bi[:])

    # ---- fold: weff[s_in, sblk*4096 + hw*64 + f]
    # f32r tile: the PSUM->SBUF copy rounds to the PE's fp32r precision,
    # which the BIR verifier requires for fp32r matmul operands.
    weff = weff_pool.tile([128, 2 * 64 * F], f32r)
    for sblk in range(2):
        for hwg in range(8):
            pw = fold_ps.tile([128, 512], f32)
            for j in range(8):
                hw = hwg * 8 + j
                for ti, (t0, tsz) in enumerate(TCH):
                    nc.tensor.matmul(
                        pw[:, j * 64:(j + 1) * 64],
                        dm_sb[ti][:, sblk * 128:(sblk + 1) * 128],
                        wt_sb[ti][:, hw * 64:(hw + 1) * 64],
                        start=(ti == 0), stop=(ti == 2),
                    )
            nc.vector.tensor_copy(
                weff[:, sblk * 4096 + hwg * 512: sblk * 4096 + (hwg + 1) * 512],
                pw[:],
            )

    # ---- main: out^T[f, b] accumulated over 128 contraction chunks
    acc = out_ps.tile([F, BC], f32)
    for gg in range(G // XG):
        xt_tile = xpool.tile([128, XG, BC], f32r)
        src = xt[gg * XG * 128:(gg + 1) * XG * 128, :].rearrange(
            "(c p) b -> p c b", p=128)
        nc.sync.dma_start(xt_tile[:], src)
        for i in range(XG):
            g = gg * XG + i
            hw, sblk = g // 2, g % 2
            nc.tensor.matmul(
                acc[:],
                weff[:, sblk * 4096 + hw * 64: sblk * 4096 + (hw + 1) * 64],
                xt_tile[:, i, :],
                start=(g == 0), stop=(g == G - 1),
            )

    # ---- epilogue: bias + LeakyReLU, store
    t1 = opool.tile([F, BC], f32)
    y = opool.tile([F, BC], f32)
    nc.vector.tensor_scalar_add(t1[:], acc[:], bias[:])
    nc.vector.scalar_tensor_tensor(
        y[:], t1[:], NEG_SLOPE, t1[:],
        op0=mybir.AluOpType.mult, op1=mybir.AluOpType.max,
    )
    nc.sync.dma_start(outT[:], y[:])


_CACHE = {}


def _get_kernel():
    if "nc" not in _CACHE:
        nc = bacc.Bacc("TRN2", target_bir_lowering=False, debug=False)
        f32 = mybir.dt.float32
        f32r = mybir.dt.float32r
        xt_d = nc.dram_tensor("xt", [G * 128, BC], f32r, kind="ExternalInput")
        wt_d = nc.dram_tensor("wt", [TDWT, 64 * F], f32, kind="ExternalInput")
        dm_d = nc.dram_tensor("dm", [TDWT, T], f32, kind="ExternalInput")
        bi_d = nc.dram_tensor("bi", [F, 1], f32, kind="ExternalInput")
        out_d = nc.dram_tensor("outT", [F, BC], f32, kind="ExternalOutput")
        with tile.TileContext(nc) as tc, ExitStack() as ctx:
            _emit(ctx, tc, xt_d.ap(), wt_d.ap(), dm_d.ap(), bi_d.ap(), out_d.ap())
        nc.compile()
        _CACHE["nc"] = nc
    return _CACHE["nc"]


def make_in_maps(x, W, b):
    dwt_m = _build_dwt_matrix()
    dm = np.ascontiguousarray(dwt_m.T).astype(np.float32)          # (276, 256)
    wt = np.ascontiguousarray(
        W[:, 0].reshape(F, TDWT, 64).transpose(1, 2, 0)
    ).reshape(TDWT, 64 * F).astype(np.float32)                     # (t, hw*64+f)
    bi = np.ascontiguousarray(b.reshape(F, 1)).astype(np.float32)
    in_maps = []
    for c in range(NCORES):
        xc = x[c * BC:(c + 1) * BC, 0].reshape(BC, T, 64)
        xt = np.ascontiguousarray(xc.transpose(2, 1, 0)).reshape(G * 128, BC)
        in_maps.append({"xt": xt, "wt": wt, "dm": dm, "bi": bi})
    return in_maps


def kernel(x, W, b, _trace=False):
    nc = _get_kernel()
    in_maps = make_in_maps(np.asarray(x), np.asarray(W), np.asarray(b))
    res = run_bass_kernel_spmd(nc, in_maps, list(range(NCORES)), trace=_trace)
    out = np.empty((B, F), np.float32)
    for c in range(NCORES):
        out[c * BC:(c + 1) * BC] = res.results[c]["outT"].T
    if _trace:
        return out, res
    return out


# revision 12
# speedup vs baseline: 3.7272x; 3.7272x over previous
"""Trainium2 kernel for DWTFeatureModel.

Model: 3-level db4 DWT along time (256 -> 276 coeffs, reflect padding) for
each of B*64 channels, then a Conv3d whose kernel spans the whole
(276, 8, 8) volume (== full contraction to 64 features), bias, LeakyReLU.

The DWT is linear, so dwt(sig) = sig @ M for a fixed (256, 276) analysis
matrix M built from the db4 filter bank. The whole model then collapses to

    out[b, f] = leaky(sum_{s,hw} x[b, s, hw] * Weff[s, hw, f] + bias[f])
    Weff[s, hw, f] = sum_t M[s, t] * W[f, t, hw]

Device kernel (per core, pure batch-data-parallel over 8 cores):
  phase 1 (fold):  Weff = M^T-contraction of the replicated conv weight,
                   computed on the tensor engine in fp32 (exact), laid out
                   directly as the stationary operands of phase 2.
  phase 2 (main):  out^T = Weff^T @ x^T as 128 accumulating matmuls
                   (K=128 each, N=256 batch columns) in fp32r, streaming
                   x from HBM (16 MB/core) in 2 MB double-buffered tiles.
  epilogue:        + bias, LeakyReLU via max(y, 0.02*y), DMA out.

Host side only shards/permutes inputs (x^T per core) and transposes the
(64, 256) per-core outputs back.
"""

from contextlib import ExitStack

import numpy as np

import concourse.bass as bass
import concourse.tile as tile
from concourse import bacc, mybir
from concourse.bass_utils import run_bass_kernel_spmd

# pywt db4 analysis filters (identical constants to the model definition)
DEC_LO = [-0.010597401784997278, 0.032883011666982945, 0.030841381835986965,
          -0.18703481171888114, -0.02798376941698385, 0.6308807679295904,
          0.7148465705525415, 0.23037781330885523]
DEC_HI = [-0.23037781330885523, 0.7148465705525415, -0.6308807679295904,
          -0.02798376941698385, 0.18703481171888114, 0.030841381835986965,
          -0.032883011666982945, -0.010597401784997278]

B, T, F, TDWT = 2048, 256, 64, 276
J, L = 3, 8
NEG_SLOPE = 0.02
NCORES = 8
BC = B // NCORES          # 256 batches per core
G = 128                   # contraction chunks of 128 (= 2 s-blocks x 64 hw)
XG = 16                   # contraction chunks per streamed x tile (2 MB)
TCH = [(0, 128), (128, 128), (256, 20)]  # t-chunks of the 276 DWT coeffs


def _build_dwt_matrix():
    """M (T, TDWT) with dwt(sig) = sig @ M, matching the reference's
    multi-level reflect-padded strided cross-correlation."""
    h_lo = np.array(DEC_LO, np.float64)[::-1]
    h_hi = np.array(DEC_HI, np.float64)[::-1]
    lo = np.eye(T, dtype=np.float64)
    his = []
    for _ in range(J):
        n = lo.shape[-1]
        outsize = (n + L - 1) // 2
        p = 2 * (outsize - 1) - n + L
        xp = np.pad(lo, ((0, 0), (p // 2, (p + 1) // 2)), mode="reflect")
        idx = np.arange(outsize)[:, None] * 2 + np.arange(L)[None, :]
        win = xp[:, idx]
        his.append(win @ h_hi)
        lo = win @ h_lo
    return np.concatenate([lo] + his, axis=-1)  # (256, 276)


def _emit(ctx, tc, xt, wt, dm, bi, outT):
    nc = tc.nc
    f32 = mybir.dt.float32
    bf16 = mybir.dt.bfloat16

    const_pool = ctx.enter_context(tc.tile_pool(name="const", bufs=1))
    weff_pool = ctx.enter_context(tc.tile_pool(name="weff", bufs=1))
    xpool = ctx.enter_context(tc.tile_pool(name="x", bufs=3))
    fold_ps = ctx.enter_context(tc.tile_pool(name="foldps", bufs=2, space="PSUM"))
    out_ps = ctx.enter_context(tc.tile_pool(name="outps", bufs=1, space="PSUM"))
    opool = ctx.enter_context(tc.tile_pool(name="osb", bufs=1))

    # ---- constants
    wt_sb, dm_sb = [], []
    for ti, (t0, tsz) in enumerate(TCH):
        w = const_pool.tile([tsz, 64 * F], bf16, tag=f"wt{ti}")
        nc.sync.dma_start(w[:], wt[t0:t0 + tsz, :])
        wt_sb.append(w)
        d = const_pool.tile([tsz, T], bf16, tag=f"dm{ti}")
        nc.sync.dma_start(d[:], dm[t0:t0 + tsz, :])
        dm_sb.append(d)
    bias = const_pool.tile([F, 1], f32, tag="bias")
    nc.sync.dma_start(bias[:], bi[:])

    # ---- fold: weff[s_in, sblk*4096 + hw*64 + f] = sum_t D[t,s] W[f,t,hw]
    # One N=512 matmul covers 8 hw x 64 f contiguous output columns.
    weff = weff_pool.tile([128, 2 * 64 * F], bf16)
    for sblk in range(2):
        for hwg in range(8):
            pw = fold_ps.tile([128, 512], f32)
            for ti, (t0, tsz) in enumerate(TCH):
                nc.tensor.matmul(
                    pw[:],
                    dm_sb[ti][:, sblk * 128:(sblk + 1) * 128],
                    wt_sb[ti][:, hwg * 512:(hwg + 1) * 512],
                    start=(ti == 0), stop=(ti == 2),
                )
            nc.vector.tensor_copy(
                weff[:, sblk * 4096 + hwg * 512: sblk * 4096 + (hwg + 1) * 512],
                pw[:],
            )

    # ---- main: out^T[f, b] accumulated over 128 contraction chunks
    acc = out_ps.tile([F, BC], f32)
    for gg in range(G // XG):
        xt_tile = xpool.tile([128, XG, BC], bf16)
        src = xt[gg * XG * 128:(gg + 1) * XG * 128, :].rearrange(
            "(c p) b -> p c b", p=128)
        nc.sync.dma_start(xt_tile[:], src)
        for i in range(XG):
            g = gg * XG + i
            hw, sblk = g // 2, g % 2
            nc.tensor.matmul(
                acc[:],
                weff[:, sblk * 4096 + hw * 64: sblk * 4096 + (hw + 1) * 64],
                xt_tile[:, i, :],
                start=(g == 0), stop=(g == G - 1),
            )

    # ---- epilogue: bias + LeakyReLU, store
    t1 = opool.tile([F, BC], f32)
    y = opool.tile([F, BC], f32)
    nc.vector.tensor_scalar_add(t1[:], acc[:], bias[:])
    nc.vector.scalar_tensor_tensor(
        y[:], t1[:], NEG_SLOPE, t1[:],
        op0=mybir.AluOpType.mult, op1=mybir.AluOpType.max,
    )
    nc.sync.dma_start(outT[:], y[:])


_CACHE = {}


def _get_kernel():
    if "nc" not in _CACHE:
        nc = bacc.Bacc("TRN2", target_bir_lowering=False, debug=False)
        f32 = mybir.dt.float32
        bf16 = mybir.dt.bfloat16
        xt_d = nc.dram_tensor("xt", [G * 128, BC], bf16, kind="ExternalInput")
        wt_d = nc.dram_tensor("wt", [TDWT, 64 * F], bf16, kind="ExternalInput")
        dm_d = nc.dram_tensor("dm", [TDWT, T], bf16, kind="ExternalInput")
        bi_d = nc.dram_tensor("bi", [F, 1], f32, kind="ExternalInput")
        out_d = nc.dram_tensor("outT", [F, BC], f32, kind="ExternalOutput")
        with tile.TileContext(nc) as tc, ExitStack() as ctx:
            _emit(ctx, tc, xt_d.ap(), wt_d.ap(), dm_d.ap(), bi_d.ap(), out_d.ap())
        nc.compile()
        _CACHE["nc"] = nc
    return _CACHE["nc"]


def make_in_maps(x, W, b):
    import ml_dtypes
    bf16 = ml_dtypes.bfloat16
    dwt_m = _build_dwt_matrix()
    dm = np.ascontiguousarray(dwt_m.T).astype(bf16)                # (276, 256)
    wt = np.ascontiguousarray(
        W[:, 0].reshape(F, TDWT, 64).transpose(1, 2, 0)
    ).reshape(TDWT, 64 * F).astype(bf16)                           # (t, hw*64+f)
    bi = np.ascontiguousarray(b.reshape(F, 1)).astype(np.float32)
    in_maps = []
    for c in range(NCORES):
        xc = x[c * BC:(c + 1) * BC, 0].reshape(BC, T, 64)
        xt = np.ascontiguousarray(
            xc.transpose(2, 1, 0).astype(bf16)).reshape(G * 128, BC)
        in_maps.append({"xt": xt, "wt": wt, "dm": dm, "bi": bi})
    return in_maps


def kernel(x, W, b, _trace=False):
    nc = _get_kernel()
    in_maps = make_in_maps(np.asarray(x), np.asarray(W), np.asarray(b))
    res = run_bass_kernel_spmd(nc, in_maps, list(range(NCORES)), trace=_trace)
    out = np.empty((B, F), np.float32)
    for c in range(NCORES):
        out[c * BC:(c + 1) * BC] = res.results[c]["outT"].T
    if _trace:
        return out, res
    return out
